# revision 6
# baseline (speedup 1.0000x reference)
"""Trainium2 Bass kernel for nn_AttentionMLP (pairwise-MLP attention + softmax).

Math (per batch b):
  hA = inputA[b] @ W1[:128]          # (K, H)
  hB = inputB[b] @ W1[128:]          # (L, H)
  scores[k, l] = sum_h relu(hA[k, h] + hB[l, h] + b1[h]) * w2[h]
  out[b, k, :] = softmax(scores[k, :])

Shapes: B=2, K=128, L=4096, D=H=128.

Distribution: pure data parallel over the (b, k) grid — core c handles
b = c // 4 and a 32-wide k block (no collectives; the softmax over L is
core-local).

Per-core device algorithm (SBUF partition axis = H):
  1. hBT = W1b.T @ inputB[b].T -> PSUM, evacuated to SBUF as bf16
     [128, 4096] in 512-col chunks (inputs pre-transposed on host so the
     contraction dim lands on partitions). Evac chunks are split across
     ACT and Pool (GpSimd) so DVE keeps its cycles for relu passes.
  2. bias[:, k] = W1a.T @ inputA[b].T + b1  (fp32 [128, 32])
  3. Per k: R_k = relu(hBT + bias[:, k]) as one [128, 4096] bf16 pass —
     statically load-balanced over THREE engines: VectorE tensor_scalar
     (add+max, 4x mode, ~1.1us), ScalarE activation (per-partition bias,
     ~3.5us) and Pool/GpSimd tensor_scalar (~5.7us).
  4. scores = w2.T @ R_k via M=32 matmuls (N=512 chunks) whose weight
     matrix is a 32-wide slice of a zeros|w2|zeros band — w2 lands in
     column 4*(k%8)+(chunk//2), so chunk c of k accumulates into PSUM
     partition 4*k + c//2, columns 512*(c%2):...  All 256 matmuls
     accumulate into ONE [128, 1024] PSUM tile (2 banks) holding the
     scores in softmax layout: partition 4k+q = l-range [1024q:1024(q+1)).
     Four matmuls run concurrently via PE col-tiling (tile_position=
     (0,32j), k's 8 apart), so PE streams ~4 cols/cycle.
  5. Softmax without max-subtraction (scores are O(1)): ScalarE exp reads
     the PSUM tile directly (this is also the PSUM evacuation) with
     accum_out producing row sums; per-k sums = quarter sums combined and
     broadcast back via tiny 0/1 matmuls; final scale (Pool) and output
     DMA per half.

Timing-loop structure: the harness measures steady-state per-iteration
time of a tc.For_i hardware loop. The loop body is traced TWICE per
For_i iteration with double-buffered (bufs=2) xbt/hbt/e_ps/psum pools so
consecutive iterations pipeline (iteration i+1's DMA + hBT production
overlap iteration i's scoring/softmax tail).
"""

import os
import sys

for _p in ("/opt/trn_rl_repo", "/root/.axon_site/_ro/trn_rl_repo"):
    if os.path.isdir(_p) and _p not in sys.path:
        sys.path.insert(0, _p)

import numpy as np
import ml_dtypes

BF = ml_dtypes.bfloat16
B, K, L, D, H = 2, 128, 4096, 128, 128
NCORES = 8
KPC = 32   # k's per core
NG = 8     # concurrency groups; group g = k's {g, 8+g, 16+g, 24+g}

import json as _json


def _env_set(name, default):
    v = os.environ.get(name)
    if v:
        return {tuple(p) if isinstance(p, list) else p for p in _json.loads(v)}
    return default


# engine assignment for the 32 relu passes (g, j); default DVE.
# NOTE: Pool (GpSimd) has no PSUM port — it may only touch SBUF.
ACT_PASSES = _env_set("KERNEL_ACT_PASSES",
                      {(0, 2), (2, 2), (4, 2), (5, 2), (6, 2)})
# passes split ACT first part / DVE rest at column ACT_SPLIT_COL
ACT_SPLIT_PASSES = _env_set("KERNEL_ACT_SPLIT", {(7, 2)})
ACT_SPLIT_COL = int(os.environ.get("KERNEL_ACT_SPLIT_COL", "1024"))
POOL_PASSES = _env_set("KERNEL_POOL_PASSES", {(1, 1), (3, 1), (5, 1), (6, 1)})
# passes whose first half runs on Pool, second half on DVE
POOL_HALF_PASSES = _env_set("KERNEL_POOL_HALF", set())
# hbt PSUM->SBUF evac chunks on ACT (rest on DVE; Pool can't read PSUM)
ACT_COPIES = _env_set("KERNEL_ACT_COPIES", {0, 1, 2, 3, 4, 5, 6, 7})
# final scale-mul engine: "pool", "vector"
MUL_ENGINE = os.environ.get("KERNEL_MUL_ENGINE", "pool")

_BUILT = None


def _build(reps=1, loop=False):
    import concourse.mybir as mybir
    import concourse.tile as tile
    from concourse import bacc

    dt = mybir.dt
    f32, bf = dt.float32, dt.bfloat16
    AF = mybir.ActivationFunctionType
    ALU = mybir.AluOpType

    nc = bacc.Bacc("TRN2", target_bir_lowering=False, debug=False,
                   enable_asserts=True)

    xbt = nc.dram_tensor("xbt", [128, L], bf, kind="ExternalInput").ap()
    xat = nc.dram_tensor("xat", [128, KPC], bf, kind="ExternalInput").ap()
    w1a = nc.dram_tensor("w1a", [128, H], bf, kind="ExternalInput").ap()
    w1b = nc.dram_tensor("w1b", [128, H], bf, kind="ExternalInput").ap()
    b1c = nc.dram_tensor("b1c", [128, 1], f32, kind="ExternalInput").ap()
    wband = nc.dram_tensor("wband", [128, 64], bf, kind="ExternalInput").ap()
    wcomb = nc.dram_tensor("wcomb", [128, KPC], f32, kind="ExternalInput").ap()
    wbcast = nc.dram_tensor("wbcast", [KPC, 128], f32, kind="ExternalInput").ap()
    out = nc.dram_tensor("out", [128, 1024], f32, kind="ExternalOutput").ap()

    with tile.TileContext(nc) as tc:
        with (
            tc.tile_pool(name="consts", bufs=1) as consts,
            tc.tile_pool(name="work", bufs=2) as work,
            tc.tile_pool(name="rpool", bufs=12) as rpool,
            tc.tile_pool(name="psum", bufs=2, space="PSUM") as psum,
            tc.tile_pool(name="epsum", bufs=2, space="PSUM") as epsum,
        ):
            w1a_sb = consts.tile([128, H], bf, tag="w1a")
            nc.sync.dma_start(w1a_sb[:], w1a)
            w1b_sb = consts.tile([128, H], bf, tag="w1b")
            nc.sync.dma_start(w1b_sb[:], w1b)
            xat_sb = consts.tile([128, KPC], bf, tag="xat")
            nc.sync.dma_start(xat_sb[:], xat)
            b1_sb = consts.tile([128, 1], f32, tag="b1")
            nc.sync.dma_start(b1_sb[:], b1c)
            wband_sb = consts.tile([128, 64], bf, tag="wband")
            nc.sync.dma_start(wband_sb[:], wband)
            wcomb_sb = consts.tile([128, KPC], f32, tag="wcomb")
            nc.sync.dma_start(wcomb_sb[:], wcomb)
            wbcast_sb = consts.tile([KPC, 128], f32, tag="wbcast")
            nc.sync.dma_start(wbcast_sb[:], wbcast)
            # dummy ACT op issued first so the ~2.7us activation-table load
            # overlaps the input DMAs instead of stalling the first real
            # ScalarE op
            warm_sb = consts.tile([128, 1], f32, tag="warm")
            nc.vector.memset(warm_sb[:], 0.0)
            # Exp anchors the exp_and_others table set, which also holds
            # Relu and Copy — one table load serves the whole kernel
            nc.scalar.activation(warm_sb[:], warm_sb[:], AF.Exp)

            args = (nc, tc, work, rpool, psum, epsum, xbt, out,
                    w1a_sb, w1b_sb, xat_sb, b1_sb, wband_sb, wcomb_sb,
                    wbcast_sb, f32, bf, AF, ALU)
            if loop and reps > 1:
                assert reps % 2 == 0, "loop reps must be even (2x unroll)"
                with tc.For_i(0, reps // 2, 1):
                    _body(*args)
                    _body(*args)
            else:
                for _rep in range(reps):
                    _body(*args)

    nc.compile()
    return nc


def _body(nc, tc, work, rpool, psum, epsum, xbt, out,
          w1a_sb, w1b_sb, xat_sb, b1_sb, wband_sb, wcomb_sb, wbcast_sb,
          f32, bf, AF, ALU):
            xbt_sb = work.tile([128, L], bf, tag="xbt")
            for c in range(8):
                nc.sync.dma_start(xbt_sb[:, 512 * c:512 * (c + 1)],
                                  xbt[:, 512 * c:512 * (c + 1)])

            # bias matrix: hAT + b1  (fp32 [128, KPC])
            ps_h = psum.tile([128, 512], f32, tag="ps")
            nc.tensor.matmul(ps_h[:, 0:KPC], lhsT=w1a_sb[:], rhs=xat_sb[:],
                             start=True, stop=True)
            bias_sb = work.tile([128, KPC], f32, tag="bias")
            nc.vector.tensor_scalar(out=bias_sb[:], in0=ps_h[:, 0:KPC],
                                    scalar1=b1_sb[:, 0:1], scalar2=None,
                                    op0=ALU.add)

            # hBT in bf16 SBUF; PSUM evac chunks split ACT/Pool/DVE
            hbt_sb = work.tile([128, L], bf, tag="hbt")
            for c in range(8):
                ps_c = psum.tile([128, 512], f32, tag="ps")
                sl = slice(512 * c, 512 * c + 512)
                nc.tensor.matmul(ps_c[:], lhsT=w1b_sb[:], rhs=xbt_sb[:, sl],
                                 start=True, stop=True)
                if c in ACT_COPIES:
                    nc.scalar.copy(hbt_sb[:, sl], ps_c[:])
                else:
                    nc.vector.tensor_copy(hbt_sb[:, sl], ps_c[:])

            # scores accumulate into one [128, 1024] PSUM tile:
            # partition 4k+q holds l-range [1024q : 1024(q+1))
            e_ps = epsum.tile([128, 1024], f32, tag="eps")

            for g in range(NG):
                rts = []
                for j in range(4):
                    k = 8 * j + g
                    rt = rpool.tile([128, L], bf, tag="r")
                    # group 0 passes split in halves: the first half only
                    # needs hbt chunks 0-3, so R production (and the PE)
                    # starts ~2us earlier in the single-shot execution
                    if (g, j) in POOL_HALF_PASSES:
                        halves = ((0, 2048, "pool"), (2048, 4096, "dve"))
                    elif (g, j) in POOL_PASSES:
                        halves = ((0, 4096, "pool"),)
                    elif (g, j) in ACT_SPLIT_PASSES:
                        halves = ((0, ACT_SPLIT_COL, "act"),
                                  (ACT_SPLIT_COL, 4096, "dve"))
                    elif (g, j) in ACT_PASSES:
                        halves = ((0, 2048, "act"), (2048, 4096, "act")) \
                            if g == 0 else ((0, 4096, "act"),)
                    else:
                        halves = ((0, 2048, "dve"), (2048, 4096, "dve")) \
                            if g == 0 else ((0, 4096, "dve"),)
                    for lo, hi, eng in halves:
                        if eng == "act":
                            nc.scalar.activation(rt[:, lo:hi],
                                                 hbt_sb[:, lo:hi], AF.Relu,
                                                 bias=bias_sb[:, k:k + 1],
                                                 scale=1.0)
                        elif eng == "pool":
                            nc.gpsimd.tensor_scalar(
                                out=rt[:, lo:hi], in0=hbt_sb[:, lo:hi],
                                scalar1=bias_sb[:, k:k + 1], scalar2=0.0,
                                op0=ALU.add, op1=ALU.max)
                        else:
                            nc.vector.tensor_scalar(
                                out=rt[:, lo:hi], in0=hbt_sb[:, lo:hi],
                                scalar1=bias_sb[:, k:k + 1], scalar2=0.0,
                                op0=ALU.add, op1=ALU.max)
                    rts.append(rt)
                # q-major: one weight slice serves 8 matmuls; in the last
                # group run all win=0 matmuls first so the exp on the first
                # PSUM bank can overlap the win=1 matmuls
                if g < NG - 1:
                    order = [(2 * q + win, j) for q in range(4)
                             for j in range(4) for win in range(2)]
                else:
                    order = ([(2 * q, j) for q in range(4) for j in range(4)]
                             + [(2 * q + 1, j) for q in range(4)
                                for j in range(4)])
                for c, j in order:
                    win = c % 2
                    v = 4 * g + c // 2  # local column for w2
                    nc.tensor.matmul(
                        e_ps[32 * j:32 * j + 32,
                             512 * win:512 * win + 512],
                        lhsT=wband_sb[:, 31 - v:63 - v],
                        rhs=rts[j][:, 512 * c:512 * c + 512],
                        start=(g == 0 and c // 2 == 0),
                        stop=(g == NG - 1 and c // 2 == 3),
                        tile_position=(0, 32 * j),
                        skip_group_check=True)

            # softmax tail; exp directly off PSUM = evacuation.
            # Two halves (by PSUM bank) so exp of bank 0 overlaps the
            # win=1 matmuls of the last group.
            e2_sb = work.tile([128, 1024], f32, tag="exp")
            s0_sb = work.tile([128, 1], f32, tag="sums0")
            s1_sb = work.tile([128, 1], f32, tag="sums1")
            nc.scalar.activation(e2_sb[:, 0:512], e_ps[:, 0:512], AF.Exp,
                                 accum_out=s0_sb[:, 0:1])
            nc.scalar.activation(e2_sb[:, 512:1024], e_ps[:, 512:1024],
                                 AF.Exp, accum_out=s1_sb[:, 0:1])
            # combine the two half-sums via PSUM accumulation; the s0 matmul
            # runs while the win=1 exp is still in flight
            ps_t = psum.tile([128, 512], f32, tag="ps")
            nc.tensor.matmul(ps_t[0:KPC, 0:1], lhsT=wcomb_sb[:],
                             rhs=s0_sb[:, 0:1], start=True, stop=False)
            nc.tensor.matmul(ps_t[0:KPC, 0:1], lhsT=wcomb_sb[:],
                             rhs=s1_sb[:, 0:1], start=False, stop=True)
            tr_sb = work.tile([KPC, 1], f32, tag="recip")
            nc.vector.reciprocal(tr_sb[:], ps_t[0:KPC, 0:1])
            ps_u = psum.tile([128, 512], f32, tag="ps")
            nc.tensor.matmul(ps_u[:, 0:1], lhsT=wbcast_sb[:], rhs=tr_sb[:],
                             start=True, stop=True)
            f_sb = work.tile([128, 1024], f32, tag="final")
            if MUL_ENGINE == "pool":
                # Pool can't read PSUM: stage the scale column in SBUF
                sc_sb = work.tile([128, 1], f32, tag="scale")
                nc.vector.tensor_copy(sc_sb[:], ps_u[:, 0:1])
                _mul, _sc = nc.gpsimd.tensor_scalar_mul, sc_sb[:, 0:1]
            else:
                _mul, _sc = nc.vector.tensor_scalar_mul, ps_u[:, 0:1]
            _mul(out=f_sb[:, 0:512], in0=e2_sb[:, 0:512], scalar1=_sc)
            nc.sync.dma_start(out[:, 0:512], f_sb[:, 0:512])
            _mul(out=f_sb[:, 512:1024], in0=e2_sb[:, 512:1024], scalar1=_sc)
            nc.sync.dma_start(out[:, 512:1024], f_sb[:, 512:1024])


def _get_built():
    global _BUILT
    if _BUILT is None:
        _BUILT = _build()
    return _BUILT


def make_in_maps(inputA, inputB, W1, b1, w2):
    wband = np.zeros((128, 64), np.float32)
    wband[:, 31] = w2
    wcomb = (np.arange(128)[:, None] // 4 == np.arange(KPC)[None, :]) \
        .astype(np.float32)
    wbcast = (np.arange(128)[None, :] // 4 == np.arange(KPC)[:, None]) \
        .astype(np.float32)
    w1a = np.ascontiguousarray(W1[:D]).astype(BF)
    w1b = np.ascontiguousarray(W1[D:]).astype(BF)
    b1c = np.ascontiguousarray(b1.reshape(128, 1)).astype(np.float32)
    wband = wband.astype(BF)
    in_maps = []
    for core in range(NCORES):
        b, kq = core // 4, core % 4
        k0 = KPC * kq
        in_maps.append({
            "xbt": np.ascontiguousarray(inputB[b].T).astype(BF),
            "xat": np.ascontiguousarray(inputA[b, k0:k0 + KPC].T).astype(BF),
            "w1a": w1a, "w1b": w1b, "b1c": b1c, "wband": wband,
            "wcomb": wcomb, "wbcast": wbcast,
        })
    return in_maps


def assemble(results):
    """results: list of 8 dicts with 'out' [128, 1024] f32."""
    full = np.empty((B, K, L), np.float32)
    for core in range(NCORES):
        b, kq = core // 4, core % 4
        full[b, KPC * kq:KPC * (kq + 1)] = \
            np.asarray(results[core]["out"]).reshape(KPC, L)
    return full


def kernel(**inputs):
    from concourse.bass_utils import run_bass_kernel_spmd

    inputA = np.asarray(inputs["inputA"], np.float32)
    inputB = np.asarray(inputs["inputB"], np.float32)
    W1 = np.asarray(inputs["W1"], np.float32)
    b1 = np.asarray(inputs["b1"], np.float32)
    w2 = np.asarray(inputs["w2"], np.float32)

    nc = _get_built()
    in_maps = make_in_maps(inputA, inputB, W1, b1, w2)
    res = run_bass_kernel_spmd(nc, in_maps, core_ids=list(range(NCORES)))
    return assemble(res.results)


# revision 9
# speedup vs baseline: 7.7632x; 7.7632x over previous
"""Trainium2 Bass kernel for nn_AttentionMLP (pairwise-MLP attention + softmax).

Math (per batch b):
  hA = inputA[b] @ W1[:128]          # (K, H)
  hB = inputB[b] @ W1[128:]          # (L, H)
  scores[k, l] = sum_h relu(hA[k, h] + hB[l, h] + b1[h]) * w2[h]
  out[b, k, :] = softmax(scores[k, :])

Shapes: B=2, K=128, L=4096, D=H=128.

Distribution: pure data parallel over the (b, k) grid — core c handles
b = c // 4 and a 32-wide k block (no collectives; the softmax over L is
core-local).

Per-core device algorithm (SBUF partition axis = H):
  1. hBT = W1b.T @ inputB[b].T -> PSUM, evacuated to SBUF as bf16
     [128, 4096] in 512-col chunks (inputs pre-transposed on host so the
     contraction dim lands on partitions). Evac chunks are split across
     ACT and Pool (GpSimd) so DVE keeps its cycles for relu passes.
  2. bias[:, k] = W1a.T @ inputA[b].T + b1  (fp32 [128, 32])
  3. Per k: R_k = relu(hBT + bias[:, k]) as one [128, 4096] bf16 pass —
     statically load-balanced over THREE engines: VectorE tensor_scalar
     (add+max, 4x mode, ~1.1us), ScalarE activation (per-partition bias,
     ~3.5us) and Pool/GpSimd tensor_scalar (~5.7us).
  4. scores = w2.T @ R_k via M=32 matmuls (N=512 chunks) whose weight
     matrix is a 32-wide slice of a zeros|w2|zeros band — w2 lands in
     column 4*(k%8)+(chunk//2), so chunk c of k accumulates into PSUM
     partition 4*k + c//2, columns 512*(c%2):...  All 256 matmuls
     accumulate into ONE [128, 1024] PSUM tile (2 banks) holding the
     scores in softmax layout: partition 4k+q = l-range [1024q:1024(q+1)).
     Four matmuls run concurrently via PE col-tiling (tile_position=
     (0,32j), k's 8 apart), so PE streams ~4 cols/cycle.
  5. Softmax without max-subtraction (scores are O(1)): ScalarE exp reads
     the PSUM tile directly (this is also the PSUM evacuation) with
     accum_out producing row sums; per-k sums = quarter sums combined and
     broadcast back via tiny 0/1 matmuls; final scale (Pool) and output
     DMA per half.

Timing-loop structure: the harness measures steady-state per-iteration
time of a tc.For_i hardware loop. The loop body is traced TWICE per
For_i iteration with double-buffered (bufs=2) xbt/hbt/e_ps/psum pools so
consecutive iterations pipeline (iteration i+1's DMA + hBT production
overlap iteration i's scoring/softmax tail).
"""

import os
import sys

for _p in ("/opt/trn_rl_repo", "/root/.axon_site/_ro/trn_rl_repo"):
    if os.path.isdir(_p) and _p not in sys.path:
        sys.path.insert(0, _p)

import numpy as np
import ml_dtypes

BF = ml_dtypes.bfloat16
B, K, L, D, H = 2, 128, 4096, 128, 128
NCORES = 8
KPC = 32   # k's per core
NG = 8     # concurrency groups; group g = k's {g, 8+g, 16+g, 24+g}

import json as _json


def _env_set(name, default):
    v = os.environ.get(name)
    if v:
        return {tuple(p) if isinstance(p, list) else p for p in _json.loads(v)}
    return default


# engine assignment for the 32 relu passes (g, j); default DVE.
# NOTE: Pool (GpSimd) is useless here — no PSUM port, and its
# tensor_scalar measured ~70us per [128,4096] pass on real HW (~50x the
# cost-model estimate), so everything stays on DVE/ACT.
ACT_PASSES = _env_set("KERNEL_ACT_PASSES",
                      {(0, 2), (2, 2), (4, 2), (5, 2), (6, 2), (7, 2)})
# passes split ACT first part / DVE rest at column ACT_SPLIT_COL
ACT_SPLIT_PASSES = _env_set("KERNEL_ACT_SPLIT", set())
ACT_SPLIT_COL = int(os.environ.get("KERNEL_ACT_SPLIT_COL", "1024"))
POOL_PASSES = _env_set("KERNEL_POOL_PASSES", set())
# passes whose first half runs on Pool, second half on DVE
POOL_HALF_PASSES = _env_set("KERNEL_POOL_HALF", set())
# hbt PSUM->SBUF evac chunks on ACT (rest on DVE; Pool can't read PSUM)
ACT_COPIES = _env_set("KERNEL_ACT_COPIES", {0, 1, 2, 3, 4, 5, 6, 7})
# final scale-mul engine: "act" (Copy+scale, reads PSUM), "vector", "pool"
MUL_ENGINE = os.environ.get("KERNEL_MUL_ENGINE", "act")

_BUILT = None


def _build(reps=1, loop=False):
    import concourse.mybir as mybir
    import concourse.tile as tile
    from concourse import bacc

    dt = mybir.dt
    f32, bf = dt.float32, dt.bfloat16
    AF = mybir.ActivationFunctionType
    ALU = mybir.AluOpType

    nc = bacc.Bacc("TRN2", target_bir_lowering=False, debug=False,
                   enable_asserts=True)

    xbt = nc.dram_tensor("xbt", [128, L], bf, kind="ExternalInput").ap()
    xat = nc.dram_tensor("xat", [128, KPC], bf, kind="ExternalInput").ap()
    w1a = nc.dram_tensor("w1a", [128, H], bf, kind="ExternalInput").ap()
    w1b = nc.dram_tensor("w1b", [128, H], bf, kind="ExternalInput").ap()
    b1c = nc.dram_tensor("b1c", [128, 1], f32, kind="ExternalInput").ap()
    wband = nc.dram_tensor("wband", [128, 64], bf, kind="ExternalInput").ap()
    wcomb = nc.dram_tensor("wcomb", [128, KPC], f32, kind="ExternalInput").ap()
    wbcast = nc.dram_tensor("wbcast", [KPC, 128], f32, kind="ExternalInput").ap()
    out = nc.dram_tensor("out", [128, 1024], f32, kind="ExternalOutput").ap()

    with tile.TileContext(nc) as tc:
        with (
            tc.tile_pool(name="consts", bufs=1) as consts,
            tc.tile_pool(name="work", bufs=2) as work,
            tc.tile_pool(name="rpool", bufs=12) as rpool,
            tc.tile_pool(name="psum", bufs=2, space="PSUM") as psum,
            tc.tile_pool(name="epsum", bufs=2, space="PSUM") as epsum,
        ):
            w1a_sb = consts.tile([128, H], bf, tag="w1a")
            nc.sync.dma_start(w1a_sb[:], w1a)
            w1b_sb = consts.tile([128, H], bf, tag="w1b")
            nc.sync.dma_start(w1b_sb[:], w1b)
            xat_sb = consts.tile([128, KPC], bf, tag="xat")
            nc.sync.dma_start(xat_sb[:], xat)
            b1_sb = consts.tile([128, 1], f32, tag="b1")
            nc.sync.dma_start(b1_sb[:], b1c)
            wband_sb = consts.tile([128, 64], bf, tag="wband")
            nc.sync.dma_start(wband_sb[:], wband)
            wcomb_sb = consts.tile([128, KPC], f32, tag="wcomb")
            nc.sync.dma_start(wcomb_sb[:], wcomb)
            wbcast_sb = consts.tile([KPC, 128], f32, tag="wbcast")
            nc.sync.dma_start(wbcast_sb[:], wbcast)
            # dummy ACT op issued first so the ~2.7us activation-table load
            # overlaps the input DMAs instead of stalling the first real
            # ScalarE op
            warm_sb = consts.tile([128, 1], f32, tag="warm")
            nc.vector.memset(warm_sb[:], 0.0)
            # Exp anchors the exp_and_others table set, which also holds
            # Relu and Copy — one table load serves the whole kernel
            nc.scalar.activation(warm_sb[:], warm_sb[:], AF.Exp)

            args = (nc, tc, work, rpool, psum, epsum, xbt, out,
                    w1a_sb, w1b_sb, xat_sb, b1_sb, wband_sb, wcomb_sb,
                    wbcast_sb, f32, bf, AF, ALU)
            if loop and reps > 1:
                assert reps % 2 == 0, "loop reps must be even (2x unroll)"
                with tc.For_i(0, reps // 2, 1):
                    _body(*args)
                    _body(*args)
            else:
                for _rep in range(reps):
                    _body(*args)

    nc.compile()
    return nc


def _body(nc, tc, work, rpool, psum, epsum, xbt, out,
          w1a_sb, w1b_sb, xat_sb, b1_sb, wband_sb, wcomb_sb, wbcast_sb,
          f32, bf, AF, ALU):
            xbt_sb = work.tile([128, L], bf, tag="xbt")
            for c in range(8):
                nc.sync.dma_start(xbt_sb[:, 512 * c:512 * (c + 1)],
                                  xbt[:, 512 * c:512 * (c + 1)])

            # bias matrix: hAT + b1  (fp32 [128, KPC])
            ps_h = psum.tile([128, 512], f32, tag="ps")
            nc.tensor.matmul(ps_h[:, 0:KPC], lhsT=w1a_sb[:], rhs=xat_sb[:],
                             start=True, stop=True)
            bias_sb = work.tile([128, KPC], f32, tag="bias")
            nc.vector.tensor_scalar(out=bias_sb[:], in0=ps_h[:, 0:KPC],
                                    scalar1=b1_sb[:, 0:1], scalar2=None,
                                    op0=ALU.add)

            # hBT in bf16 SBUF; PSUM evac chunks split ACT/Pool/DVE
            hbt_sb = work.tile([128, L], bf, tag="hbt")
            for c in range(8):
                ps_c = psum.tile([128, 512], f32, tag="ps")
                sl = slice(512 * c, 512 * c + 512)
                nc.tensor.matmul(ps_c[:], lhsT=w1b_sb[:], rhs=xbt_sb[:, sl],
                                 start=True, stop=True)
                if c in ACT_COPIES:
                    nc.scalar.copy(hbt_sb[:, sl], ps_c[:])
                else:
                    nc.vector.tensor_copy(hbt_sb[:, sl], ps_c[:])

            # scores accumulate into one [128, 1024] PSUM tile:
            # partition 4k+q holds l-range [1024q : 1024(q+1))
            e_ps = epsum.tile([128, 1024], f32, tag="eps")

            for g in range(NG):
                rts = []
                for j in range(4):
                    k = 8 * j + g
                    rt = rpool.tile([128, L], bf, tag="r")
                    # group 0 passes split in halves: the first half only
                    # needs hbt chunks 0-3, so R production (and the PE)
                    # starts ~2us earlier in the single-shot execution
                    if (g, j) in POOL_HALF_PASSES:
                        halves = ((0, 2048, "pool"), (2048, 4096, "dve"))
                    elif (g, j) in POOL_PASSES:
                        halves = ((0, 4096, "pool"),)
                    elif (g, j) in ACT_SPLIT_PASSES:
                        halves = ((0, ACT_SPLIT_COL, "act"),
                                  (ACT_SPLIT_COL, 4096, "dve"))
                    elif (g, j) in ACT_PASSES:
                        halves = ((0, 2048, "act"), (2048, 4096, "act")) \
                            if g == 0 else ((0, 4096, "act"),)
                    else:
                        halves = ((0, 2048, "dve"), (2048, 4096, "dve")) \
                            if g == 0 else ((0, 4096, "dve"),)
                    for lo, hi, eng in halves:
                        if eng == "act":
                            nc.scalar.activation(rt[:, lo:hi],
                                                 hbt_sb[:, lo:hi], AF.Relu,
                                                 bias=bias_sb[:, k:k + 1],
                                                 scale=1.0)
                        elif eng == "pool":
                            nc.gpsimd.tensor_scalar(
                                out=rt[:, lo:hi], in0=hbt_sb[:, lo:hi],
                                scalar1=bias_sb[:, k:k + 1], scalar2=0.0,
                                op0=ALU.add, op1=ALU.max)
                        else:
                            nc.vector.tensor_scalar(
                                out=rt[:, lo:hi], in0=hbt_sb[:, lo:hi],
                                scalar1=bias_sb[:, k:k + 1], scalar2=0.0,
                                op0=ALU.add, op1=ALU.max)
                    rts.append(rt)
                # q-major: one weight slice serves 8 matmuls; in the last
                # group run all win=0 matmuls first so the exp on the first
                # PSUM bank can overlap the win=1 matmuls
                if g < NG - 1:
                    order = [(2 * q + win, j) for q in range(4)
                             for j in range(4) for win in range(2)]
                else:
                    order = ([(2 * q, j) for q in range(4) for j in range(4)]
                             + [(2 * q + 1, j) for q in range(4)
                                for j in range(4)])
                for c, j in order:
                    win = c % 2
                    v = 4 * g + c // 2  # local column for w2
                    nc.tensor.matmul(
                        e_ps[32 * j:32 * j + 32,
                             512 * win:512 * win + 512],
                        lhsT=wband_sb[:, 31 - v:63 - v],
                        rhs=rts[j][:, 512 * c:512 * c + 512],
                        start=(g == 0 and c // 2 == 0),
                        stop=(g == NG - 1 and c // 2 == 3),
                        tile_position=(0, 32 * j),
                        skip_group_check=True)

            # softmax tail; exp directly off PSUM = evacuation.
            # Two halves (by PSUM bank) so exp of bank 0 overlaps the
            # win=1 matmuls of the last group.
            e2_sb = work.tile([128, 1024], f32, tag="exp")
            s0_sb = work.tile([128, 1], f32, tag="sums0")
            s1_sb = work.tile([128, 1], f32, tag="sums1")
            nc.scalar.activation(e2_sb[:, 0:512], e_ps[:, 0:512], AF.Exp,
                                 accum_out=s0_sb[:, 0:1])
            nc.scalar.activation(e2_sb[:, 512:1024], e_ps[:, 512:1024],
                                 AF.Exp, accum_out=s1_sb[:, 0:1])
            # combine the two half-sums via PSUM accumulation; the s0 matmul
            # runs while the win=1 exp is still in flight
            ps_t = psum.tile([128, 512], f32, tag="ps")
            nc.tensor.matmul(ps_t[0:KPC, 0:1], lhsT=wcomb_sb[:],
                             rhs=s0_sb[:, 0:1], start=True, stop=False)
            nc.tensor.matmul(ps_t[0:KPC, 0:1], lhsT=wcomb_sb[:],
                             rhs=s1_sb[:, 0:1], start=False, stop=True)
            tr_sb = work.tile([KPC, 1], f32, tag="recip")
            nc.vector.reciprocal(tr_sb[:], ps_t[0:KPC, 0:1])
            ps_u = psum.tile([128, 512], f32, tag="ps")
            nc.tensor.matmul(ps_u[:, 0:1], lhsT=wbcast_sb[:], rhs=tr_sb[:],
                             start=True, stop=True)
            f_sb = work.tile([128, 1024], f32, tag="final")
            if MUL_ENGINE == "act":
                # activation scale AP must be SBUF: stage it via DVE
                sc_sb = work.tile([128, 1], f32, tag="scale")
                nc.vector.tensor_copy(sc_sb[:], ps_u[:, 0:1])

                def _mul(out, in0, scalar1):
                    nc.scalar.activation(out, in0, AF.Copy, scale=scalar1)
                _sc = sc_sb[:, 0:1]
            elif MUL_ENGINE == "pool":
                # Pool can't read PSUM: stage the scale column in SBUF
                sc_sb = work.tile([128, 1], f32, tag="scale")
                nc.vector.tensor_copy(sc_sb[:], ps_u[:, 0:1])
                _mul, _sc = nc.gpsimd.tensor_scalar_mul, sc_sb[:, 0:1]
            else:
                _mul, _sc = nc.vector.tensor_scalar_mul, ps_u[:, 0:1]
            _mul(out=f_sb[:, 0:512], in0=e2_sb[:, 0:512], scalar1=_sc)
            nc.sync.dma_start(out[:, 0:512], f_sb[:, 0:512])
            _mul(out=f_sb[:, 512:1024], in0=e2_sb[:, 512:1024], scalar1=_sc)
            nc.sync.dma_start(out[:, 512:1024], f_sb[:, 512:1024])


def _get_built():
    global _BUILT
    if _BUILT is None:
        _BUILT = _build()
    return _BUILT


def make_in_maps(inputA, inputB, W1, b1, w2):
    wband = np.zeros((128, 64), np.float32)
    wband[:, 31] = w2
    wcomb = (np.arange(128)[:, None] // 4 == np.arange(KPC)[None, :]) \
        .astype(np.float32)
    wbcast = (np.arange(128)[None, :] // 4 == np.arange(KPC)[:, None]) \
        .astype(np.float32)
    w1a = np.ascontiguousarray(W1[:D]).astype(BF)
    w1b = np.ascontiguousarray(W1[D:]).astype(BF)
    b1c = np.ascontiguousarray(b1.reshape(128, 1)).astype(np.float32)
    wband = wband.astype(BF)
    in_maps = []
    for core in range(NCORES):
        b, kq = core // 4, core % 4
        k0 = KPC * kq
        in_maps.append({
            "xbt": np.ascontiguousarray(inputB[b].T).astype(BF),
            "xat": np.ascontiguousarray(inputA[b, k0:k0 + KPC].T).astype(BF),
            "w1a": w1a, "w1b": w1b, "b1c": b1c, "wband": wband,
            "wcomb": wcomb, "wbcast": wbcast,
        })
    return in_maps


def assemble(results):
    """results: list of 8 dicts with 'out' [128, 1024] f32."""
    full = np.empty((B, K, L), np.float32)
    for core in range(NCORES):
        b, kq = core // 4, core % 4
        full[b, KPC * kq:KPC * (kq + 1)] = \
            np.asarray(results[core]["out"]).reshape(KPC, L)
    return full


def kernel(**inputs):
    from concourse.bass_utils import run_bass_kernel_spmd

    inputA = np.asarray(inputs["inputA"], np.float32)
    inputB = np.asarray(inputs["inputB"], np.float32)
    W1 = np.asarray(inputs["W1"], np.float32)
    b1 = np.asarray(inputs["b1"], np.float32)
    w2 = np.asarray(inputs["w2"], np.float32)

    nc = _get_built()
    in_maps = make_in_maps(inputA, inputB, W1, b1, w2)
    res = run_bass_kernel_spmd(nc, in_maps, core_ids=list(range(NCORES)))
    return assemble(res.results)


# revision 11
# speedup vs baseline: 8.4874x; 1.0933x over previous
"""Trainium2 Bass kernel for nn_AttentionMLP (pairwise-MLP attention + softmax).

Math (per batch b):
  hA = inputA[b] @ W1[:128]          # (K, H)
  hB = inputB[b] @ W1[128:]          # (L, H)
  scores[k, l] = sum_h relu(hA[k, h] + hB[l, h] + b1[h]) * w2[h]
  out[b, k, :] = softmax(scores[k, :])

Shapes: B=2, K=128, L=4096, D=H=128.

Distribution: pure data parallel over the (b, k) grid — core c handles
b = c // 4 and a 32-wide k block (no collectives; the softmax over L is
core-local).

Per-core device algorithm (SBUF partition axis = H):
  1. hBT = W1b.T @ inputB[b].T -> PSUM, evacuated to SBUF as bf16
     [128, 4096] in 512-col chunks (inputs pre-transposed on host so the
     contraction dim lands on partitions).
  2. bias[:, k] = W1a.T @ inputA[b].T + b1  (fp32 [128, 32])
  3. Per k: R_k = relu(hBT + bias[:, k]) as one [128, 4096] bf16 pass —
     load-balanced DVE (tensor_scalar add+max, 4x mode, ~1.1us measured)
     vs ACT (activation Relu with per-partition bias, ~4.1us measured).
     GpSimd/Pool is useless here (~70us/pass measured, no PSUM port).
  4. scores = w2.T @ R_k via M=32 matmuls (N=512 chunks) whose weight
     matrix is a 32-wide slice of a zeros|w2|zeros band — w2 lands in
     column 4*(k%8)+(chunk//2), so chunk c of k accumulates into PSUM
     partition 4*k + c//2, columns 512*(c%2):...  All 256 matmuls
     accumulate into ONE [128, 1024] PSUM tile (2 banks) holding the
     scores in softmax layout: partition 4k+q = l-range [1024q:1024(q+1)).
     Four matmuls run concurrently via PE col-tiling (tile_position=
     (0,32j), k's 8 apart), so PE streams ~4 cols/cycle.
  5. Softmax without max-subtraction (scores are O(1)): ScalarE exp reads
     the PSUM tile directly (this is also the PSUM evacuation) with
     accum_out producing row sums; per-k sums = quarter sums combined and
     broadcast back via tiny 0/1 matmuls; final scale via ACT Copy+scale
     and per-half output DMA.

Timing-loop structure (software-pipelined, measured steady state):
each loop half-body processes logical iteration i as
  [groups 0-1 of i | softmax tail of i-1 | groups 2-3 | produce i+1
   (xbt DMA, bias, hBT matmuls+evacs) | groups 4-7]
so every engine queue is gap-free: the serial softmax-tail chain of
iteration i-1 and the hBT production for i+1 both overlap iteration i's
relu/scoring work. Tiles double-buffer via tag rotation (bufs=2 pools,
two half-body calls per For_i trace). The pre/post halves outside the
loop cancel in the (T2-T1)/(N2-N1) timing difference.
"""

import os
import sys

for _p in ("/opt/trn_rl_repo", "/root/.axon_site/_ro/trn_rl_repo"):
    if os.path.isdir(_p) and _p not in sys.path:
        sys.path.insert(0, _p)

import numpy as np
import ml_dtypes

BF = ml_dtypes.bfloat16
B, K, L, D, H = 2, 128, 4096, 128, 128
NCORES = 8
KPC = 32   # k's per core
NG = 8     # concurrency groups; group g = k's {g, 8+g, 16+g, 24+g}

import json as _json


def _env_set(name, default):
    v = os.environ.get(name)
    if v:
        return {tuple(p) if isinstance(p, list) else p for p in _json.loads(v)}
    return default


# engine assignment for the 32 relu passes (g, j); default DVE
ACT_PASSES = _env_set("KERNEL_ACT_PASSES",
                      {(0, 2), (2, 2), (4, 2), (6, 2), (7, 2)})
# passes split ACT [0:COL] / DVE [COL:4096]
ACT_SPLIT_PASSES = _env_set("KERNEL_ACT_SPLIT", {(5, 2)})
ACT_SPLIT_COL = int(os.environ.get("KERNEL_ACT_SPLIT_COL", "2048"))
# hbt PSUM->SBUF evac chunks on ACT (rest on DVE)
ACT_COPIES = _env_set("KERNEL_ACT_COPIES", {0, 1, 2, 3, 4, 5, 6, 7})
# final scale-mul engine: "act" (Copy+scale) or "vector"
MUL_ENGINE = os.environ.get("KERNEL_MUL_ENGINE", "act")

_BUILT = None


class _Ctx:
    def __init__(self, nc, tc, pools, aps, types):
        self.nc, self.tc = nc, tc
        (self.work, self.rpool, self.psum, self.epsum) = pools
        (self.xbt, self.out, self.w1a_sb, self.w1b_sb, self.xat_sb,
         self.b1_sb, self.wband_sb, self.wcomb_sb, self.wbcast_sb) = aps
        (self.f32, self.bf, self.AF, self.ALU) = types


def _emit_produce(cx):
    """xbt DMA + bias matrix + hBT matmuls/evacs for one logical iter."""
    nc, f32, bf, ALU = cx.nc, cx.f32, cx.bf, cx.ALU
    xbt_sb = cx.work.tile([128, L], bf, tag="xbt")
    for c in range(8):
        nc.sync.dma_start(xbt_sb[:, 512 * c:512 * (c + 1)],
                          cx.xbt[:, 512 * c:512 * (c + 1)])

    ps_h = cx.psum.tile([128, 512], f32, tag="ps")
    nc.tensor.matmul(ps_h[:, 0:KPC], lhsT=cx.w1a_sb[:], rhs=cx.xat_sb[:],
                     start=True, stop=True)
    bias_sb = cx.work.tile([128, KPC], f32, tag="bias")
    nc.vector.tensor_scalar(out=bias_sb[:], in0=ps_h[:, 0:KPC],
                            scalar1=cx.b1_sb[:, 0:1], scalar2=None,
                            op0=ALU.add)

    hbt_sb = cx.work.tile([128, L], bf, tag="hbt")
    for c in range(8):
        ps_c = cx.psum.tile([128, 512], f32, tag="ps")
        sl = slice(512 * c, 512 * c + 512)
        nc.tensor.matmul(ps_c[:], lhsT=cx.w1b_sb[:], rhs=xbt_sb[:, sl],
                         start=True, stop=True)
        if c in ACT_COPIES:
            nc.scalar.copy(hbt_sb[:, sl], ps_c[:])
        else:
            nc.vector.tensor_copy(hbt_sb[:, sl], ps_c[:])
    return {"hbt": hbt_sb, "bias": bias_sb}


def _emit_group(cx, cur, g, e_ps, split_g0=False):
    """4 relu passes + 32 scoring matmuls for concurrency group g."""
    nc, f32, bf, AF, ALU = cx.nc, cx.f32, cx.bf, cx.AF, cx.ALU
    hbt_sb, bias_sb = cur["hbt"], cur["bias"]
    rts = []
    for j in range(4):
        k = 8 * j + g
        rt = cx.rpool.tile([128, L], bf, tag="r")
        if (g, j) in ACT_SPLIT_PASSES:
            parts = ((0, ACT_SPLIT_COL, "act"), (ACT_SPLIT_COL, 4096, "dve"))
        elif (g, j) in ACT_PASSES:
            parts = ((0, 2048, "act"), (2048, 4096, "act"))
        elif split_g0 and g == 0:
            # halves so the first half only needs hbt chunks 0-3
            # (single-shot startup)
            parts = ((0, 2048, "dve"), (2048, 4096, "dve"))
        else:
            parts = ((0, 4096, "dve"),)
        for lo, hi, eng in parts:
            if eng == "act":
                nc.scalar.activation(rt[:, lo:hi], hbt_sb[:, lo:hi], AF.Relu,
                                     bias=bias_sb[:, k:k + 1], scale=1.0)
            else:
                nc.vector.tensor_scalar(
                    out=rt[:, lo:hi], in0=hbt_sb[:, lo:hi],
                    scalar1=bias_sb[:, k:k + 1], scalar2=0.0,
                    op0=ALU.add, op1=ALU.max)
        rts.append(rt)
    # q-major: one weight slice serves 8 matmuls; in the last group run
    # all win=0 matmuls first so the exp on the first PSUM bank can
    # overlap the win=1 matmuls
    if g < NG - 1:
        order = [(2 * q + win, j) for q in range(4)
                 for j in range(4) for win in range(2)]
    else:
        order = ([(2 * q, j) for q in range(4) for j in range(4)]
                 + [(2 * q + 1, j) for q in range(4) for j in range(4)])
    for c, j in order:
        win = c % 2
        v = 4 * g + c // 2  # local column for w2
        nc.tensor.matmul(
            e_ps[32 * j:32 * j + 32, 512 * win:512 * win + 512],
            lhsT=cx.wband_sb[:, 31 - v:63 - v],
            rhs=rts[j][:, 512 * c:512 * c + 512],
            start=(g == 0 and c // 2 == 0),
            stop=(g == NG - 1 and c // 2 == 3),
            tile_position=(0, 32 * j),
            skip_group_check=True)


def _emit_tail(cx, st):
    """Softmax tail draining scoring state `st` (exp, sums, scale, out)."""
    nc, f32, AF = cx.nc, cx.f32, cx.AF
    e_ps = st["eps"]
    e2_sb = cx.work.tile([128, 1024], f32, tag="exp")
    s0_sb = cx.work.tile([128, 1], f32, tag="sums0")
    s1_sb = cx.work.tile([128, 1], f32, tag="sums1")
    nc.scalar.activation(e2_sb[:, 0:512], e_ps[:, 0:512], AF.Exp,
                         accum_out=s0_sb[:, 0:1])
    nc.scalar.activation(e2_sb[:, 512:1024], e_ps[:, 512:1024],
                         AF.Exp, accum_out=s1_sb[:, 0:1])
    # combine the two half-sums via PSUM accumulation
    ps_t = cx.psum.tile([128, 512], f32, tag="ps")
    nc.tensor.matmul(ps_t[0:KPC, 0:1], lhsT=cx.wcomb_sb[:],
                     rhs=s0_sb[:, 0:1], start=True, stop=False)
    nc.tensor.matmul(ps_t[0:KPC, 0:1], lhsT=cx.wcomb_sb[:],
                     rhs=s1_sb[:, 0:1], start=False, stop=True)
    tr_sb = cx.work.tile([KPC, 1], f32, tag="recip")
    nc.vector.reciprocal(tr_sb[:], ps_t[0:KPC, 0:1])
    ps_u = cx.psum.tile([128, 512], f32, tag="ps")
    nc.tensor.matmul(ps_u[:, 0:1], lhsT=cx.wbcast_sb[:], rhs=tr_sb[:],
                     start=True, stop=True)
    f_sb = cx.work.tile([128, 1024], f32, tag="final")
    if MUL_ENGINE == "act":
        sc_sb = cx.work.tile([128, 1], f32, tag="scale")
        nc.vector.tensor_copy(sc_sb[:], ps_u[:, 0:1])

        def _mul(out, in0, scalar1):
            nc.scalar.activation(out, in0, AF.Copy, scale=scalar1)
        _sc = sc_sb[:, 0:1]
    else:
        _mul, _sc = nc.vector.tensor_scalar_mul, ps_u[:, 0:1]
    _mul(out=f_sb[:, 0:512], in0=e2_sb[:, 0:512], scalar1=_sc)
    nc.sync.dma_start(cx.out[:, 0:512], f_sb[:, 0:512])
    _mul(out=f_sb[:, 512:1024], in0=e2_sb[:, 512:1024], scalar1=_sc)
    nc.sync.dma_start(cx.out[:, 512:1024], f_sb[:, 512:1024])


def _emit_half(cx, cur, prev_st, produce_next):
    """Pipelined half-body: passes+scoring of `cur`, tail of `prev_st`,
    optional produce of the next logical iteration. Returns (state, nxt)."""
    f32 = cx.f32
    e_ps = cx.epsum.tile([128, 1024], f32, tag="eps")
    _emit_group(cx, cur, 0, e_ps)
    _emit_group(cx, cur, 1, e_ps)
    if prev_st is not None:
        _emit_tail(cx, prev_st)
    _emit_group(cx, cur, 2, e_ps)
    _emit_group(cx, cur, 3, e_ps)
    nxt = _emit_produce(cx) if produce_next else None
    for g in range(4, NG):
        _emit_group(cx, cur, g, e_ps)
    return {"eps": e_ps}, nxt


def _body_straight(cx):
    """Single-shot body: produce + groups (g0 split for startup) + tail."""
    cur = _emit_produce(cx)
    f32 = cx.f32
    e_ps = cx.epsum.tile([128, 1024], f32, tag="eps")
    for g in range(NG):
        _emit_group(cx, cur, g, e_ps, split_g0=True)
    _emit_tail(cx, {"eps": e_ps})


def _build(reps=1, loop=False):
    import concourse.mybir as mybir
    import concourse.tile as tile
    from concourse import bacc

    dt = mybir.dt
    f32, bf = dt.float32, dt.bfloat16
    AF = mybir.ActivationFunctionType
    ALU = mybir.AluOpType

    nc = bacc.Bacc("TRN2", target_bir_lowering=False, debug=False,
                   enable_asserts=True)

    xbt = nc.dram_tensor("xbt", [128, L], bf, kind="ExternalInput").ap()
    xat = nc.dram_tensor("xat", [128, KPC], bf, kind="ExternalInput").ap()
    w1a = nc.dram_tensor("w1a", [128, H], bf, kind="ExternalInput").ap()
    w1b = nc.dram_tensor("w1b", [128, H], bf, kind="ExternalInput").ap()
    b1c = nc.dram_tensor("b1c", [128, 1], f32, kind="ExternalInput").ap()
    wband = nc.dram_tensor("wband", [128, 64], bf, kind="ExternalInput").ap()
    wcomb = nc.dram_tensor("wcomb", [128, KPC], f32, kind="ExternalInput").ap()
    wbcast = nc.dram_tensor("wbcast", [KPC, 128], f32, kind="ExternalInput").ap()
    out = nc.dram_tensor("out", [128, 1024], f32, kind="ExternalOutput").ap()

    with tile.TileContext(nc) as tc:
        with (
            tc.tile_pool(name="consts", bufs=1) as consts,
            tc.tile_pool(name="work", bufs=2) as work,
            tc.tile_pool(name="rpool", bufs=12) as rpool,
            tc.tile_pool(name="psum", bufs=2, space="PSUM") as psum,
            tc.tile_pool(name="epsum", bufs=2, space="PSUM") as epsum,
        ):
            w1a_sb = consts.tile([128, H], bf, tag="w1a")
            nc.sync.dma_start(w1a_sb[:], w1a)
            w1b_sb = consts.tile([128, H], bf, tag="w1b")
            nc.sync.dma_start(w1b_sb[:], w1b)
            xat_sb = consts.tile([128, KPC], bf, tag="xat")
            nc.sync.dma_start(xat_sb[:], xat)
            b1_sb = consts.tile([128, 1], f32, tag="b1")
            nc.sync.dma_start(b1_sb[:], b1c)
            wband_sb = consts.tile([128, 64], bf, tag="wband")
            nc.sync.dma_start(wband_sb[:], wband)
            wcomb_sb = consts.tile([128, KPC], f32, tag="wcomb")
            nc.sync.dma_start(wcomb_sb[:], wcomb)
            wbcast_sb = consts.tile([KPC, 128], f32, tag="wbcast")
            nc.sync.dma_start(wbcast_sb[:], wbcast)
            # dummy ACT op issued first so the ~2.7us activation-table load
            # overlaps the input DMAs; Exp anchors the exp_and_others table
            # set which also holds Relu and Copy
            warm_sb = consts.tile([128, 1], f32, tag="warm")
            nc.vector.memset(warm_sb[:], 0.0)
            nc.scalar.activation(warm_sb[:], warm_sb[:], AF.Exp)

            cx = _Ctx(nc, tc,
                      (work, rpool, psum, epsum),
                      (xbt, out, w1a_sb, w1b_sb, xat_sb, b1_sb, wband_sb,
                       wcomb_sb, wbcast_sb),
                      (f32, bf, AF, ALU))

            if loop and reps > 1:
                assert reps % 2 == 0 and reps >= 4, \
                    "loop path needs even reps >= 4"
                n = reps // 2
                cur0 = _emit_produce(cx)                    # logical 0
                st_a, cur1 = _emit_half(cx, cur0, None, True)
                with tc.For_i(0, n - 1, 1):
                    st_b, cur0b = _emit_half(cx, cur1, st_a, True)
                    st_a2, cur1b = _emit_half(cx, cur0b, st_b, True)
                st_last, _ = _emit_half(cx, cur1, st_a, False)
                _emit_tail(cx, st_last)
            else:
                for _rep in range(reps):
                    _body_straight(cx)

    nc.compile()
    return nc


def _get_built():
    global _BUILT
    if _BUILT is None:
        _BUILT = _build()
    return _BUILT


def make_in_maps(inputA, inputB, W1, b1, w2):
    wband = np.zeros((128, 64), np.float32)
    wband[:, 31] = w2
    wcomb = (np.arange(128)[:, None] // 4 == np.arange(KPC)[None, :]) \
        .astype(np.float32)
    wbcast = (np.arange(128)[None, :] // 4 == np.arange(KPC)[:, None]) \
        .astype(np.float32)
    w1a = np.ascontiguousarray(W1[:D]).astype(BF)
    w1b = np.ascontiguousarray(W1[D:]).astype(BF)
    b1c = np.ascontiguousarray(b1.reshape(128, 1)).astype(np.float32)
    wband = wband.astype(BF)
    in_maps = []
    for core in range(NCORES):
        b, kq = core // 4, core % 4
        k0 = KPC * kq
        in_maps.append({
            "xbt": np.ascontiguousarray(inputB[b].T).astype(BF),
            "xat": np.ascontiguousarray(inputA[b, k0:k0 + KPC].T).astype(BF),
            "w1a": w1a, "w1b": w1b, "b1c": b1c, "wband": wband,
            "wcomb": wcomb, "wbcast": wbcast,
        })
    return in_maps


def assemble(results):
    """results: list of 8 dicts with 'out' [128, 1024] f32."""
    full = np.empty((B, K, L), np.float32)
    for core in range(NCORES):
        b, kq = core // 4, core % 4
        full[b, KPC * kq:KPC * (kq + 1)] = \
            np.asarray(results[core]["out"]).reshape(KPC, L)
    return full


def kernel(**inputs):
    from concourse.bass_utils import run_bass_kernel_spmd

    inputA = np.asarray(inputs["inputA"], np.float32)
    inputB = np.asarray(inputs["inputB"], np.float32)
    W1 = np.asarray(inputs["W1"], np.float32)
    b1 = np.asarray(inputs["b1"], np.float32)
    w2 = np.asarray(inputs["w2"], np.float32)

    nc = _get_built()
    in_maps = make_in_maps(inputA, inputB, W1, b1, w2)
    res = run_bass_kernel_spmd(nc, in_maps, core_ids=list(range(NCORES)))
    return assemble(res.results)
